# revision 1
# baseline (speedup 1.0000x reference)
"""Trainium2 Bass kernel for the PRADA GCN encoder (3x GCNConv message passing).

Math (matching the jax reference):
    src/dst = edges + self loops;  deg = indegree(dst);  dinv = rsqrt(deg)
    conv(x, W, b) = dinv_d * ((sum_{e: dst=d} dinv_src * x_src) @ W) + b
    h      = tanh(conv(x, W1, b1))
    mean   = conv(h, Wm, bm);  logvar = conv(h, Wv, bv)
    z      = noise * exp(0.5*logvar) + mean

Strategy (8 NeuronCores, single SPMD NEFF):
  - Destination nodes sharded contiguously across cores (12544 rows/core).
  - Per core, edges are grouped host-side into K=128-slot "groups"; each group
    targets one 32-destination PSUM window of one 128-dst tile.  A one-hot
    selection matrix S [128 slots, 32 dsts] (int8 from host, cast to fp32
    on-chip) turns the segment-sum into PSUM-accumulating matmuls:
        psum[win] += S_g.T @ gathered_msgs_g
  - Messages are fetched with gpsimd.dma_gather (int16 indices), so the
    source array is processed in 4 chunks of <=25088 rows.
  - The group skeleton (#groups per (tile, window, chunk)) is made uniform
    across cores by taking the max over the 8 cores, so one NEFF serves all
    cores; per-core variation lives in the idx / S input data (padded slots
    gather row 0 with a zero S column).
  - dinv_src is folded into the gather source (xs = dinv*x, hs = dinv*h);
    dinv_dst is applied after aggregation.  Projections W1 / [Wm|Wv] are
    applied per 128-dst tile after a PE transpose of the aggregate.
  - Between conv1 and conv2/3 the h-shards are AllGathered.
"""

import math
import numpy as np

# ----------------------------------------------------------------- constants
TILE_D = 128     # destinations per tile (PSUM partition dim)
WIN = 32         # destinations per PSUM window (col-group granularity)
NWIN = TILE_D // WIN
K = 128          # slots per group (matmul contraction dim)
MAX_CHUNK_ROWS = 32000   # int16 gather index limit (32767)

# problem config (graded problem; kernel.py must be self-contained)
N_NODES = 100000
N_EDGES = 1200000
IN_DIM, HID_DIM, LAT_DIM = 128, 64, 32
N_CORES = 8
GMAX = 8         # max groups (1024 idxs) per dma_gather call
ST_TILES = 6     # tiles per supertile (each aggregation tile owns a full PSUM bank)
BANK_F32 = 512   # fp32 elements per 2KB PSUM bank


def _ceil_div(a, b):
    return -(-a // b)


# ============================================================ preprocessing
class Prep:
    pass


def preprocess(edge_index, n_nodes, cores, st_tiles=ST_TILES, n_chunks=None):
    """Build the uniform group skeleton + per-core idx/S tensors."""
    p = Prep()
    src = edge_index[0].astype(np.int64)
    dst = edge_index[1].astype(np.int64)

    npad = _ceil_div(n_nodes, cores * TILE_D) * cores * TILE_D
    dpc = npad // cores
    tiles_pc = dpc // TILE_D
    if n_chunks is None:
        n_chunks = max(1, _ceil_div(npad, MAX_CHUNK_ROWS))
    chunk = _ceil_div(npad, n_chunks)
    assert chunk <= 32767
    n_st = _ceil_div(tiles_pc, st_tiles)

    # degree includes the self loop (+1); pad nodes get deg 1
    deg = np.bincount(dst, minlength=npad).astype(np.float32)
    deg[:n_nodes] += 1.0
    deg[n_nodes:] = 1.0
    dinv = (1.0 / np.sqrt(deg)).astype(np.float32)

    core_id = dst // dpc
    dloc = dst % dpc
    t = dloc // TILE_D
    w = (dloc % TILE_D) // WIN
    wcol = dloc % WIN
    c = src // chunk
    srcl = (src % chunk).astype(np.int16)

    # counts per (core, t, w, c) -> uniform skeleton G = ceil(max_cores/K)
    key = ((core_id * tiles_pc + t) * NWIN + w) * n_chunks + c
    counts = np.bincount(key, minlength=cores * tiles_pc * NWIN * n_chunks)
    counts = counts.reshape(cores, tiles_pc, NWIN, n_chunks)
    G = _ceil_div(counts.max(axis=0), K).astype(np.int64)  # [tiles_pc, NWIN, n_chunks]
    # every (t, w) needs >=1 group so its PSUM window gets written (start=True)
    empty_tw = G.sum(axis=2) == 0
    G[:, :, 0][empty_tw] = 1

    # enumerate groups in emission order: st -> chunk -> tile -> window -> g
    slot_off = np.zeros((tiles_pc, NWIN, n_chunks), np.int64)
    group_tile = []
    group_win = []
    segments = []  # per (st, c): dict
    gidx = 0
    soff = 0
    for s in range(n_st):
        ts = range(s * st_tiles, min((s + 1) * st_tiles, tiles_pc))
        for cc in range(n_chunks):
            g_lo, s_lo = gidx, soff
            for tt in ts:
                for ww in range(NWIN):
                    gg = int(G[tt, ww, cc])
                    slot_off[tt, ww, cc] = soff
                    group_tile.extend([tt] * gg)
                    group_win.extend([ww] * gg)
                    gidx += gg
                    soff += gg * K
            segments.append(dict(st=s, chunk=cc, g_lo=g_lo, g_hi=gidx,
                                 s_lo=s_lo, s_hi=soff))
    n_groups, n_slots = gidx, soff
    group_tile = np.asarray(group_tile, np.int64)
    group_win = np.asarray(group_win, np.int64)

    # first/last group flags per (t, w) in emission order
    first = np.zeros(n_groups, bool)
    last = np.zeros(n_groups, bool)
    seen = {}
    for g in range(n_groups):
        kk = (int(group_tile[g]), int(group_win[g]))
        if kk not in seen:
            first[g] = True
        seen[kk] = g
    for kk, g in seen.items():
        last[g] = True

    # ------- per-core slot assignment (vectorized rank within (core,t,w,c))
    order = np.lexsort((c, w, t, core_id))
    key_sorted = key[order]
    starts = np.r_[0, np.flatnonzero(np.diff(key_sorted)) + 1]
    group_start_of = np.zeros(len(key_sorted), np.int64)
    group_start_of[starts] = starts
    np.maximum.accumulate(group_start_of, out=group_start_of)
    rank = np.arange(len(key_sorted)) - group_start_of

    slot = slot_off[t[order], w[order], c[order]] + rank
    corev = core_id[order]

    idx16 = np.zeros((cores, n_slots), np.int16)
    idx16[corev, slot] = srcl[order]
    s8 = np.zeros((cores, 128, n_groups * 32), np.int8)
    s8[corev, slot % K, (slot // K) * 32 + wcol[order]] = 1

    # wrapped gather-index layout: idx i -> [i%16, i//16], replicated x8 rows
    wrapped = idx16.reshape(cores, -1, 16).transpose(0, 2, 1)  # [cores,16,S/16]
    wrapped = np.tile(wrapped, (1, 8, 1)).copy()               # [cores,128,S/16]

    p.npad, p.dpc, p.tiles_pc, p.n_st = npad, dpc, tiles_pc, n_st
    p.n_chunks, p.chunk, p.st_tiles = n_chunks, chunk, st_tiles
    p.n_groups, p.n_slots = n_groups, n_slots
    p.group_tile, p.group_win = group_tile, group_win
    p.group_first, p.group_last = first, last
    p.segments = segments
    p.dinv = dinv
    p.idx_wrapped = wrapped
    p.s8 = s8
    p.cores = cores
    return p


# ============================================================ bass program
def build_program(p, in_dim, hid_dim, lat_dim, has_b1, has_bmv):
    import concourse.bacc as bacc
    import concourse.mybir as mybir
    import concourse.tile as tile
    from concourse.library_config import mlp
    from concourse.masks import make_identity

    f32 = mybir.dt.float32
    nc = bacc.Bacc("TRN2", target_bir_lowering=False, debug=False,
                   num_devices=p.cores)

    xs_t = nc.dram_tensor("xs", [p.npad, in_dim], f32, kind="ExternalInput")
    idx_t = nc.dram_tensor("idx", [128, p.n_slots // 16], mybir.dt.int16,
                           kind="ExternalInput")
    s8_t = nc.dram_tensor("s8", [128, p.n_groups * 32], mybir.dt.int8,
                          kind="ExternalInput")
    dinv_t = nc.dram_tensor("dinv", [128, p.tiles_pc], f32, kind="ExternalInput")
    w1_t = nc.dram_tensor("w1", [in_dim, hid_dim], f32, kind="ExternalInput")
    wmv_t = nc.dram_tensor("wmv", [hid_dim, 2 * lat_dim], f32, kind="ExternalInput")
    b1b_t = nc.dram_tensor("b1b", [128, hid_dim], f32, kind="ExternalInput")
    bmvb_t = nc.dram_tensor("bmvb", [128, 2 * lat_dim], f32, kind="ExternalInput")
    noise_t = nc.dram_tensor("noise", [p.dpc, lat_dim], f32, kind="ExternalInput")
    xso_t = nc.dram_tensor("xs_own", [p.dpc, in_dim], f32, kind="ExternalInput")

    mean_t = nc.dram_tensor("mean", [p.dpc, lat_dim], f32, kind="ExternalOutput")
    logvar_t = nc.dram_tensor("logvar", [p.dpc, lat_dim], f32, kind="ExternalOutput")
    z_t = nc.dram_tensor("z", [p.dpc, lat_dim], f32, kind="ExternalOutput")

    segs_by_st = {}
    for seg in p.segments:
        segs_by_st.setdefault(seg["st"], []).append(seg)

    with tile.TileContext(nc) as tc:
        with (
            tc.tile_pool(name="const", bufs=1) as cpool,
            tc.tile_pool(name="sb", bufs=2) as sb,
            tc.tile_pool(name="ep", bufs=2) as ep,
            tc.tile_pool(name="ps", bufs=1, space="PSUM") as ps,
            tc.tile_pool(name="pse", bufs=1, space="PSUM") as pse,
            tc.tile_pool(name="dram", bufs=1, space="DRAM") as dram,
        ):
            nc.gpsimd.load_library(mlp)
            ident = cpool.tile([128, 128], f32)
            make_identity(nc, ident[:])
            w1_sb = cpool.tile([in_dim, hid_dim], f32)
            nc.sync.dma_start(w1_sb[:], w1_t[:])
            wmv_sb = cpool.tile([hid_dim, 2 * lat_dim], f32)
            nc.sync.dma_start(wmv_sb[:], wmv_t[:])
            dinv_sb = cpool.tile([128, p.tiles_pc], f32)
            nc.sync.dma_start(dinv_sb[:], dinv_t[:])
            if has_b1:
                b1b_sb = cpool.tile([128, hid_dim], f32)
                nc.sync.dma_start(b1b_sb[:], b1b_t[:])
            if has_bmv:
                bmvb_sb = cpool.tile([128, 2 * lat_dim], f32)
                nc.sync.dma_start(bmvb_sb[:], bmvb_t[:])

            hs_shard = dram.tile([p.dpc, hid_dim], f32)
            hs_full = dram.tile([p.npad, hid_dim], f32)

            def agg_pass(elem, src_ap_fn, epilogue):
                """One gather+aggregate pass over all supertiles."""
                for s in range(p.n_st):
                    t0 = s * p.st_tiles
                    nt = min(p.st_tiles, p.tiles_pc - t0)
                    agg_ps = ps.tile([128, nt * BANK_F32], f32, tag="agg")
                    subsegs = []
                    for seg in segs_by_st[s]:
                        for ga in range(seg["g_lo"], seg["g_hi"], GMAX):
                            gb = min(ga + GMAX, seg["g_hi"])
                            sa = seg["s_lo"] + (ga - seg["g_lo"]) * K
                            subsegs.append((seg["chunk"], ga, gb, sa))
                    for cc, ga, gb, sa in subsegs:
                        ng = gb - ga
                        nsl = ng * K
                        idx_sb = sb.tile([128, nsl // 16], mybir.dt.int16,
                                         tag="idx")
                        nc.sync.dma_start(
                            idx_sb[:],
                            idx_t[:, sa // 16: (sa + nsl) // 16])
                        s8_sb = sb.tile([128, ng * 32], mybir.dt.int8, tag="s8")
                        nc.sync.dma_start(
                            s8_sb[:], s8_t[:, ga * 32: gb * 32])
                        s32_sb = sb.tile([128, ng * 32], f32, tag="s32")
                        nc.vector.tensor_copy(s32_sb[:], s8_sb[:])
                        msgs = sb.tile([128, ng * elem], f32, tag=f"msgs{elem}")
                        msgs3 = msgs[:].rearrange("p (g e) -> p g e", g=ng)
                        r0 = cc * p.chunk
                        r1 = min(r0 + p.chunk, p.npad)
                        nc.gpsimd.dma_gather(
                            msgs3, src_ap_fn(r0, r1), idx_sb[:],
                            nsl, nsl, elem)
                        for g in range(ga, gb):
                            gq = g - ga
                            tl = int(p.group_tile[g]) - t0
                            ww = int(p.group_win[g])
                            nc.tensor.matmul(
                                agg_ps[32 * ww: 32 * (ww + 1),
                                       tl * BANK_F32: tl * BANK_F32 + elem],
                                s32_sb[:, gq * 32: (gq + 1) * 32],
                                msgs3[:, gq, :],
                                start=bool(p.group_first[g]),
                                stop=bool(p.group_last[g]),
                                tile_position=(0, 32 * ww),
                                skip_group_check=True,
                            )
                    for tl in range(nt):
                        epilogue(t0 + tl,
                                 agg_ps[:, tl * BANK_F32: tl * BANK_F32 + elem])

            # ---------------- pass 1: conv1 -> hs ----------------
            def epi1(t, agg_slice):
                dv = dinv_sb[:, t: t + 1]
                agg_sb = ep.tile([128, in_dim], f32, tag="e1agg")
                nc.scalar.copy(agg_sb[:], agg_slice)
                xso_sb = ep.tile([128, in_dim], f32, tag="e1xso")
                nc.sync.dma_start(
                    xso_sb[:], xso_t[t * TILE_D: (t + 1) * TILE_D, :])
                nc.vector.tensor_tensor(agg_sb[:], agg_sb[:], xso_sb[:],
                                        mybir.AluOpType.add)
                aggT_ps = pse.tile([128, 128], f32, tag="eT")
                nc.tensor.transpose(aggT_ps[:in_dim, :], agg_sb[:], ident[:])
                aggT_sb = ep.tile([in_dim, 128], f32, tag="e1Ts")
                nc.scalar.copy(aggT_sb[:], aggT_ps[:in_dim, :])
                h_ps = pse.tile([128, hid_dim], f32, tag="eo")
                nc.tensor.matmul(h_ps[:], aggT_sb[:], w1_sb[:],
                                 start=True, stop=True)
                hs_sb = ep.tile([128, hid_dim], f32, tag="e1hs")
                if has_b1:
                    tmp = ep.tile([128, hid_dim], f32, tag="e1tmp")
                    nc.vector.tensor_scalar(tmp[:], h_ps[:], dv, None,
                                            mybir.AluOpType.mult)
                    nc.vector.tensor_tensor(tmp[:], tmp[:], b1b_sb[:],
                                            mybir.AluOpType.add)
                    nc.scalar.activation(hs_sb[:], tmp[:],
                                         mybir.ActivationFunctionType.Tanh)
                else:
                    nc.scalar.activation(hs_sb[:], h_ps[:],
                                         mybir.ActivationFunctionType.Tanh,
                                         scale=dv)
                nc.vector.tensor_scalar(hs_sb[:], hs_sb[:], dv, None,
                                        mybir.AluOpType.mult)
                nc.sync.dma_start(
                    hs_shard[t * TILE_D: (t + 1) * TILE_D, :], hs_sb[:])

            agg_pass(in_dim, lambda r0, r1: xs_t[r0:r1, :], epi1)

            nc.gpsimd.collective_compute(
                "AllGather",
                mybir.AluOpType.bypass,
                replica_groups=[list(range(p.cores))],
                ins=[hs_shard[:]],
                outs=[hs_full[:]],
            )

            # ---------------- pass 2: conv2/3 -> mean/logvar/z ----------------
            def epi2(t, agg_slice):
                dv = dinv_sb[:, t: t + 1]
                agg_sb = ep.tile([128, hid_dim], f32, tag="e2agg")
                nc.scalar.copy(agg_sb[:], agg_slice)
                hso_sb = ep.tile([128, hid_dim], f32, tag="e2hso")
                nc.sync.dma_start(
                    hso_sb[:], hs_shard[t * TILE_D: (t + 1) * TILE_D, :])
                nc.vector.tensor_tensor(agg_sb[:], agg_sb[:], hso_sb[:],
                                        mybir.AluOpType.add)
                aggT_ps = pse.tile([128, 128], f32, tag="eT")
                nc.tensor.transpose(aggT_ps[:hid_dim, :], agg_sb[:], ident[:])
                aggT_sb = ep.tile([hid_dim, 128], f32, tag="e2Ts")
                nc.scalar.copy(aggT_sb[:], aggT_ps[:hid_dim, :])
                mlv_ps = pse.tile([128, 2 * lat_dim], f32, tag="eo")
                nc.tensor.matmul(mlv_ps[:], aggT_sb[:], wmv_sb[:],
                                 start=True, stop=True)
                mlv_sb = ep.tile([128, 2 * lat_dim], f32, tag="e2mlvs")
                nc.vector.tensor_scalar(mlv_sb[:], mlv_ps[:], dv, None,
                                        mybir.AluOpType.mult)
                if has_bmv:
                    nc.vector.tensor_tensor(mlv_sb[:], mlv_sb[:], bmvb_sb[:],
                                            mybir.AluOpType.add)
                rows = slice(t * TILE_D, (t + 1) * TILE_D)
                nc.sync.dma_start(mean_t[rows, :], mlv_sb[:, :lat_dim])
                nc.sync.dma_start(logvar_t[rows, :], mlv_sb[:, lat_dim:])
                ev_sb = ep.tile([128, lat_dim], f32, tag="e2ev")
                nc.scalar.activation(ev_sb[:], mlv_sb[:, lat_dim:],
                                     mybir.ActivationFunctionType.Exp,
                                     scale=0.5)
                noise_sb = ep.tile([128, lat_dim], f32, tag="e2n")
                nc.sync.dma_start(noise_sb[:], noise_t[rows, :])
                z_sb = ep.tile([128, lat_dim], f32, tag="e2z")
                nc.vector.tensor_tensor(z_sb[:], noise_sb[:], ev_sb[:],
                                        mybir.AluOpType.mult)
                nc.vector.tensor_tensor(z_sb[:], z_sb[:], mlv_sb[:, :lat_dim],
                                        mybir.AluOpType.add)
                nc.sync.dma_start(z_t[rows, :], z_sb[:])

            agg_pass(hid_dim, lambda r0, r1: hs_full[r0:r1, :], epi2)

    nc.compile()
    return nc


# ============================================================ host driver
def make_inputs(p, x, W1, b1, Wm, bm, Wv, bv, noise, in_dim, hid_dim, lat_dim):
    n = x.shape[0]
    xs = np.zeros((p.npad, in_dim), np.float32)
    xs[:n] = x * p.dinv[:n, None]
    noise_pad = np.zeros((p.npad, lat_dim), np.float32)
    noise_pad[:n] = noise
    wmv = np.concatenate([Wm, Wv], axis=1).astype(np.float32)
    b1b = np.tile(np.asarray(b1, np.float32)[None, :], (128, 1))
    bmvb = np.tile(np.concatenate([bm, bv]).astype(np.float32)[None, :], (128, 1))
    dinv_tiles = np.zeros((p.cores, 128, p.tiles_pc), np.float32)
    for cc in range(p.cores):
        dv = p.dinv[cc * p.dpc: (cc + 1) * p.dpc]
        dinv_tiles[cc] = dv.reshape(p.tiles_pc, TILE_D).T

    in_maps = []
    for cc in range(p.cores):
        in_maps.append({
            "xs": xs,
            "idx": p.idx_wrapped[cc],
            "s8": p.s8[cc],
            "dinv": dinv_tiles[cc],
            "w1": np.asarray(W1, np.float32),
            "wmv": wmv,
            "b1b": b1b,
            "bmvb": bmvb,
            "noise": noise_pad[cc * p.dpc: (cc + 1) * p.dpc],
            "xs_own": xs[cc * p.dpc: (cc + 1) * p.dpc],
        })
    return in_maps


def run(x, edge_index, W1, b1, Wm, bm, Wv, bv, noise,
        cores=N_CORES, st_tiles=ST_TILES, n_chunks=None, trace=False):
    from concourse.bass_utils import run_bass_kernel_spmd

    n, in_dim = x.shape
    hid_dim = W1.shape[1]
    lat_dim = Wm.shape[1]
    p = preprocess(np.asarray(edge_index, np.int64), n, cores,
                   st_tiles=st_tiles, n_chunks=n_chunks)
    has_b1 = bool(np.any(np.asarray(b1) != 0))
    has_bmv = bool(np.any(np.asarray(bm) != 0) or np.any(np.asarray(bv) != 0))
    nc = build_program(p, in_dim, hid_dim, lat_dim, has_b1, has_bmv)
    in_maps = make_inputs(p, np.asarray(x, np.float32), W1, b1, Wm, bm, Wv, bv,
                          np.asarray(noise, np.float32),
                          in_dim, hid_dim, lat_dim)
    res = run_bass_kernel_spmd(nc, in_maps, core_ids=list(range(cores)),
                               trace=trace)
    mean = np.concatenate([r["mean"] for r in res.results], axis=0)[:n]
    logvar = np.concatenate([r["logvar"] for r in res.results], axis=0)[:n]
    z = np.concatenate([r["z"] for r in res.results], axis=0)[:n]
    return (z, mean, logvar), res


def kernel(x, edge_index, W1, b1, Wm, bm, Wv, bv, noise):
    (z, mean, logvar), _ = run(np.asarray(x), np.asarray(edge_index),
                               np.asarray(W1), np.asarray(b1),
                               np.asarray(Wm), np.asarray(bm),
                               np.asarray(Wv), np.asarray(bv),
                               np.asarray(noise))
    return (z, mean, logvar)

